# revision 4
# baseline (speedup 1.0000x reference)
"""FourierLayer TRN2 kernel: folded DFT -> top-6 mask -> folded inverse DFT.

Contract: kernel(input_tensor=(8,2048,512) f32) -> (8,2048,512) f32.
Each of the 8 NeuronCores processes one batch element (data-parallel over
batch; no cross-core communication).

Math (per core, T=2048, D=512, top-6 of 1023 rfft bins):
  Time-fold (host): xs[t']=x[t']+x[2048-t'], xd[t']=x[t']-x[2048-t']
  halves the forward DFT:  Re_k = sum_t' CF[t',k] xs[t'] (+(-1)^k x[1024]),
  Im_k = sum_t' SF[t',k] xd[t'].  Forward runs 3 bf16 hi/lo products
  (Ch@xh + Ch@xl + Cl@xh) for ~fp32 magnitudes (exact top-6 selection).
  Frequency columns are ordered [k=1..512, k=1023..513, k=1024(zero)] so
  the inverse frequency-fold A+/- = R2[0:512] +/- R2[512:1024] (B-/+ for
  I2) is an aligned DVE add; even/odd output rows then come from two
  half-size inverse transforms (bf16 hi-only matrices).

Raw bass with manual semaphores (TileContext auto-sync emits >2 sync
commands per instruction, which this toolchain's walrus rejects). All DMAs
go through gpsimd/SWDGE: each transfer increments the DMA semaphore once
per SDMA engine (16 total), so cumulative thresholds are sound.
"""

from contextlib import ExitStack

import numpy as np
import ml_dtypes

import concourse.bass as bass
import concourse.mybir as mybir

BF16 = mybir.dt.bfloat16
F32 = mybir.dt.float32
AF = mybir.ActivationFunctionType
ALU = mybir.AluOpType

T = 2048          # time length
D = 512           # channels
KF = 1024         # padded frequency count (col c -> k=perm[c]; col KF-1 zero)
NKC = KF // 128   # 8 freq chunks
NDC = D // 128    # 4 channel chunks
TOPK = 6
NFC = 9           # forward folded time chunks for Re (8 + boundary)
NIC = 16          # inverse groups: (t' chunk 0..7) x (parity)
RSI = 2           # inverse stream ring slots

# ---- semaphore schedules (python-side bookkeeping) ----
# s_dma (units of 16, gpsimd program order):
#   d0 xsig, d1 cfh, d2 cfl, d3 xdsig, d4 sfh, d5 sfl, d6 iv0, d7 iv1,
#   d8..d11 theta rows, then interleaved iv_j (j>=2) / out_i:
#   iv_j at index 8+2(j-2)+4 = 2j+8, out_i at index 13+2i; total 42.
# s_pe:  fwd Re groups 1..25; Im groups 26..49; transposes 50..81;
#        ones-bcast 82; inverse groups 83..98
# s_act: Re evicts 1..8; Im evicts 9..16; transpose copies 17..48;
#        thb 49; out evicts 50..65
# s_dve: mag 1..8; max8 9..12; mask groups 13..20; folds 21..24
# s_pool: ones 1; identity 2


def build_kernel(nc: bass.Bass):
    xsig = nc.dram_tensor("xsig", (17 * 128, D), BF16, kind="ExternalInput")
    xdsig = nc.dram_tensor("xdsig", (16 * 128, D), BF16, kind="ExternalInput")
    cfh = nc.dram_tensor("cfh", (NFC * 128, KF), BF16, kind="ExternalInput")
    cfl = nc.dram_tensor("cfl", (8 * 128, KF), BF16, kind="ExternalInput")
    sfh = nc.dram_tensor("sfh", (8 * 128, KF), BF16, kind="ExternalInput")
    sfl = nc.dram_tensor("sfl", (8 * 128, KF), BF16, kind="ExternalInput")
    # inverse blocks: group g=(tc*2+par): [p, m*128+u] = M[(m%4)*128+p,
    # tc*128+u] with M = CI(par) for m<4, SI(par) for m>=4
    ivp = nc.dram_tensor("ivp", (NIC, 128, 8 * 128), BF16, kind="ExternalInput")
    out = nc.dram_tensor("out", (T, D), F32, kind="ExternalOutput")

    with ExitStack() as ctx:
        def sb(name, shape, dtype):
            return ctx.enter_context(nc.sbuf_tensor(name, shape, dtype))

        xs_sb = sb("xs_sb", [128, 17 * D], BF16)
        xd_sb = sb("xd_sb", [128, 16 * D], BF16)
        cfh_sb = sb("cfh_sb", [128, NFC * KF], BF16)
        cfl_sb = sb("cfl_sb", [128, 8 * KF], BF16)
        sfh_sb = sb("sfh_sb", [128, 8 * KF], BF16)
        sfl_sb = sb("sfl_sb", [128, 8 * KF], BF16)
        iv_sb = sb("iv_sb", [128, RSI * 8 * 128], BF16)
        r2 = sb("r2", [128, NKC * D], F32)
        i2 = sb("i2", [128, NKC * D], F32)
        mag = sb("mag", [128, NKC * D], F32)
        mag_t = sb("mag_t", [128, NDC * KF], F32)
        ap_h = sb("ap_h", [128, 4 * D], BF16)
        am_h = sb("am_h", [128, 4 * D], BF16)
        bp_h = sb("bp_h", [128, 4 * D], BF16)
        bm_h = sb("bm_h", [128, 4 * D], BF16)
        m8 = sb("m8", [128, NDC * 8], F32)
        trows = [sb(f"trow{i}", [1, 128], F32) for i in range(NDC)]
        thb = sb("thb", [128, D], F32)
        ones = sb("ones", [1, 128], F32)
        ident = sb("ident", [128, 128], F32)
        msk = sb("msk", [128, D], F32)
        sqt = sb("sqt", [128, D], F32)
        ot_sb = sb("ot_sb", [128, 2 * D], F32)
        banks = [ctx.enter_context(nc.psum_tensor(f"pb{i}", [128, D], F32))
                 for i in range(8)]
        pb5 = banks[5]
        s_dma = ctx.enter_context(nc.semaphore())
        s_pe = ctx.enter_context(nc.semaphore())
        s_act = ctx.enter_context(nc.semaphore())
        s_dve = ctx.enter_context(nc.semaphore())
        s_pool = ctx.enter_context(nc.semaphore())
        block = ctx.enter_context(nc.Block())

        # forward matmul groups: (matrix_sb, n_chunks_base, rhs_chunk_offset)
        # Re: hh (cfh x xs_h, 9 chunks incl boundary), hl (cfh x xs_l),
        #     lh (cfl x xs_h)
        # Im: same over (sfh/sfl, xd_h/xd_l)

        @block.gpsimd
        def _(gpsimd):
            gpsimd.memset(ones[:], 1.0).then_inc(s_pool, 1)
            gpsimd.memset(ident[:], 0.0)
            gpsimd.drain()
            nc.gpsimd.affine_select(
                out=ident[:], in_=ident[:],
                compare_op=ALU.not_equal, fill=1.0, base=0,
                pattern=[[-1, 128]], channel_multiplier=1,
            ).then_inc(s_pool, 1)
            # bulk loads (each +16 on completion, any order; cumulative
            # thresholds cover prefixes)
            gpsimd.dma_start(
                xs_sb[:, :],
                xsig[:].rearrange("(a p) d -> p a d", p=128)).then_inc(s_dma, 16)
            gpsimd.dma_start(
                cfh_sb[:, :],
                cfh[:].rearrange("(a p) c -> p a c", p=128)).then_inc(s_dma, 16)
            gpsimd.dma_start(
                cfl_sb[:, :],
                cfl[:].rearrange("(a p) c -> p a c", p=128)).then_inc(s_dma, 16)
            gpsimd.dma_start(
                xd_sb[:, :],
                xdsig[:].rearrange("(a p) d -> p a d", p=128)).then_inc(s_dma, 16)
            gpsimd.dma_start(
                sfh_sb[:, :],
                sfh[:].rearrange("(a p) c -> p a c", p=128)).then_inc(s_dma, 16)
            gpsimd.dma_start(
                sfl_sb[:, :],
                sfl[:].rearrange("(a p) c -> p a c", p=128)).then_inc(s_dma, 16)
            for j in range(RSI):
                gpsimd.dma_start(
                    iv_sb[:, (j % RSI) * KF:(j % RSI + 1) * KF],
                    ivp[j, :, :]).then_inc(s_dma, 16)
            # theta rows: (128,1) column -> (1,128) row via DMA
            gpsimd.wait_ge(s_dve, 12)
            for dc in range(NDC):
                gpsimd.dma_start(
                    trows[dc][:, :],
                    m8[:, dc * 8 + TOPK - 1: dc * 8 + TOPK]).then_inc(s_dma, 16)
            # remaining inverse streams interleaved with output stores
            out_v = out[:].rearrange("(a two) d -> two a d", two=2)
            for j in range(RSI, NIC + 2):
                if j < NIC:
                    gpsimd.wait_ge(s_pe, 83 + (j - RSI))
                    gpsimd.dma_start(
                        iv_sb[:, (j % RSI) * KF:(j % RSI + 1) * KF],
                        ivp[j, :, :]).then_inc(s_dma, 16)
                if j >= 2:
                    g = j - 2
                    par, tc = g % 2, g // 2
                    gpsimd.wait_ge(s_act, 50 + g)
                    gpsimd.dma_start(
                        out_v[par, tc * 128:(tc + 1) * 128, :],
                        ot_sb[:, (g % 2) * D:(g % 2 + 1) * D],
                    ).then_inc(s_dma, 16)
            gpsimd.wait_ge(s_dma, 42 * 16)

        @block.tensor
        def _(tensor):
            # forward DFT: Re (25 groups) then Im (24 groups)
            fwd = [
                # (mat_sb, mat_chunks, rhs_sb, rhs_chunk0, n, dma_wait)
                (cfh_sb, xs_sb, 0, 9, 2),    # hh incl boundary
                (cfh_sb, xs_sb, 9, 8, 2),    # hl
                (cfl_sb, xs_sb, 0, 8, 3),    # lh
                (sfh_sb, xd_sb, 0, 8, 5),    # Im hh
                (sfh_sb, xd_sb, 8, 8, 5),    # Im hl
                (sfl_sb, xd_sb, 0, 8, 6),    # Im lh
            ]
            g = 0            # global forward group counter (s_pe value = g+1)
            for pi, (mat, rhs, r0, n, dw) in enumerate(fwd):
                comp_first = pi in (0, 3)
                comp_last_g = 24 if pi < 3 else 48
                tensor.wait_ge(s_dma, dw * 16)
                if pi == 3:
                    tensor.wait_ge(s_act, 8)   # Re banks evicted
                for tc in range(n):
                    first = (g in (0, 25))
                    last = (g in (24, 48))
                    rc = rhs[:, (r0 + tc) * D:(r0 + tc + 1) * D]
                    for kc in range(NKC):
                        mm = nc.tensor.matmul(
                            banks[kc][:],
                            mat[:, tc * KF + kc * 128: tc * KF + (kc + 1) * 128],
                            rc, start=first, stop=last)
                        if kc == NKC - 1:
                            mm.then_inc(s_pe, 1)
                    g += 1
            # mag transposes (d-major so each dc finishes contiguously)
            tensor.wait_ge(s_pool, 2)
            for dc in range(NDC):
                for kc in range(NKC):
                    i = dc * NKC + kc
                    tensor.wait_ge(s_dve, kc + 1)
                    tensor.wait_ge(s_act, 13 + i if i >= 4 else 9 + i)
                    nc.tensor.transpose(
                        banks[i % 4][:, 0:128],
                        mag[:, kc * D + dc * 128: kc * D + (dc + 1) * 128],
                        ident[:]).then_inc(s_pe, 1)
            # ones-broadcast (fp32, exact): trow rows -> thb psum (bank5)
            tensor.wait_ge(s_dma, 12 * 16)  # theta row DMAs done
            for dc in range(NDC):
                mm = nc.tensor.matmul(pb5[:, dc * 128:(dc + 1) * 128],
                                      ones[:], trows[dc][:],
                                      start=(dc == 0), stop=(dc == NDC - 1))
                if dc == NDC - 1:
                    mm.then_inc(s_pe, 1)
            # inverse DFT: 16 groups (t' chunk x parity)
            tensor.wait_ge(s_dve, 24)  # folds done
            for g in range(NIC):
                par = g % 2
                tensor.wait_ge(
                    s_dma, 16 * (7 + g) if g < RSI else 16 * (9 + 2 * g))
                if g >= 4:
                    tensor.wait_ge(s_act, 46 + g)  # bank evicted
                bank = banks[g % 4]
                sl0 = (g % RSI) * KF
                ca, cb = (ap_h, bp_h) if par == 0 else (am_h, bm_h)
                for jc in range(4):
                    nc.tensor.matmul(
                        bank[:], iv_sb[:, sl0 + jc * 128: sl0 + (jc + 1) * 128],
                        ca[:, jc * D:(jc + 1) * D],
                        start=(jc == 0), stop=False)
                    mm = nc.tensor.matmul(
                        bank[:],
                        iv_sb[:, sl0 + (4 + jc) * 128: sl0 + (5 + jc) * 128],
                        cb[:, jc * D:(jc + 1) * D],
                        start=False, stop=(jc == 3))
                    if jc == 3:
                        mm.then_inc(s_pe, 1)

        @block.scalar
        def _(scalar):
            # forward evictions; x2 scale folds the conjugate doubling
            scalar.wait_ge(s_pe, 25)
            for kc in range(NKC):
                nc.scalar.activation(r2[:, kc * D:(kc + 1) * D], banks[kc][:],
                                     AF.Copy, scale=2.0).then_inc(s_act, 1)
            scalar.wait_ge(s_pe, 49)
            for kc in range(NKC):
                nc.scalar.activation(i2[:, kc * D:(kc + 1) * D], banks[kc][:],
                                     AF.Copy, scale=2.0).then_inc(s_act, 1)
            # transpose copies
            for dc in range(NDC):
                for kc in range(NKC):
                    i = dc * NKC + kc
                    scalar.wait_ge(s_pe, 50 + i)
                    nc.scalar.activation(
                        mag_t[:, dc * KF + kc * 128: dc * KF + (kc + 1) * 128],
                        banks[i % 4][:, 0:128], AF.Copy).then_inc(s_act, 1)
            # thb copy
            scalar.wait_ge(s_pe, 82)
            nc.scalar.activation(thb[:], pb5[:], AF.Copy).then_inc(s_act, 1)
            # inverse evictions
            for g in range(NIC):
                scalar.wait_ge(s_pe, 83 + g)
                if g >= 2:
                    # out-DMA (g-2) completes at 16*(13+2*(g-2)+1)
                    scalar.wait_ge(s_dma, 16 * (10 + 2 * g))
                nc.scalar.activation(
                    ot_sb[:, (g % 2) * D:(g % 2 + 1) * D],
                    banks[g % 4][:], AF.Copy).then_inc(s_act, 1)

        @block.vector
        def _(vector):
            # magnitudes
            for kc in range(NKC):
                vector.wait_ge(s_act, 9 + kc)
                dsl = slice(kc * D, (kc + 1) * D)
                nc.vector.tensor_tensor(mag[:, dsl], r2[:, dsl], r2[:, dsl],
                                        ALU.mult)
                nc.vector.tensor_tensor(sqt[:], i2[:, dsl], i2[:, dsl],
                                        ALU.mult)
                nc.vector.tensor_tensor(mag[:, dsl], mag[:, dsl], sqt[:],
                                        ALU.add).then_inc(s_dve, 1)
            # top-8 + 6th-largest per channel
            for dc in range(NDC):
                vector.wait_ge(s_act, 24 + dc * 8)
                nc.vector.max(out=m8[:, dc * 8:(dc + 1) * 8],
                              in_=mag_t[:, dc * KF:(dc + 1) * KF]).then_inc(s_dve, 1)
            # mask + apply (in place)
            vector.wait_ge(s_act, 49)
            for kc in range(NKC):
                dsl = slice(kc * D, (kc + 1) * D)
                nc.vector.tensor_tensor(msk[:], mag[:, dsl], thb[:], ALU.is_ge)
                nc.vector.tensor_tensor(r2[:, dsl], r2[:, dsl], msk[:],
                                        ALU.mult)
                nc.vector.tensor_tensor(i2[:, dsl], i2[:, dsl], msk[:],
                                        ALU.mult).then_inc(s_dve, 1)
            # frequency folds (bf16 out), halves = [128, 4*D] slices
            H = 4 * D
            nc.vector.tensor_tensor(ap_h[:], r2[:, 0:H], r2[:, H:2 * H],
                                    ALU.add).then_inc(s_dve, 1)
            nc.vector.tensor_tensor(am_h[:], r2[:, 0:H], r2[:, H:2 * H],
                                    ALU.subtract).then_inc(s_dve, 1)
            nc.vector.tensor_tensor(bp_h[:], i2[:, 0:H], i2[:, H:2 * H],
                                    ALU.subtract).then_inc(s_dve, 1)
            nc.vector.tensor_tensor(bm_h[:], i2[:, 0:H], i2[:, H:2 * H],
                                    ALU.add).then_inc(s_dve, 1)


# ---------------- host side ----------------

_BF = ml_dtypes.bfloat16


def _split_hilo(a32):
    hi = a32.astype(_BF)
    lo = (a32.astype(np.float32) - hi.astype(np.float32)).astype(_BF)
    return hi, lo


def _perm():
    p = np.empty(KF, dtype=np.int64)
    p[0:512] = np.arange(1, 513)
    p[512:1023] = np.arange(1023, 512, -1)
    p[1023] = 1024
    return p


def _make_constants():
    perm = _perm()
    tp = np.arange(1024, dtype=np.float64)[:, None]
    kk = perm[None, :].astype(np.float64)
    ang = 2.0 * np.pi * tp * kk / T
    CF = np.cos(ang)
    SF = -np.sin(ang)
    CF[:, KF - 1] = 0.0
    SF[:, KF - 1] = 0.0
    SF[0, :] = 0.0
    ch, cl = _split_hilo(CF.astype(np.float32))
    sh, sl = _split_hilo(SF.astype(np.float32))
    bnd = np.cos(np.pi * perm)
    bnd[KF - 1] = 0.0
    cfh_np = np.zeros((NFC * 128, KF), dtype=_BF)
    cfh_np[0:1024] = ch
    cfh_np[1024] = bnd.astype(_BF)
    cfh_np[1025] = bnd.astype(_BF)

    jj = np.arange(512, dtype=np.float64)[:, None] + 1.0
    te = 2.0 * np.arange(1024, dtype=np.float64)[None, :]
    mats = {}
    for par, toff in ((0, 0.0), (1, 1.0)):
        a = 2.0 * np.pi * jj * (te + toff) / T
        mats[(par, 0)] = np.cos(a).astype(np.float32).astype(_BF)
        mats[(par, 1)] = (-np.sin(a)).astype(np.float32).astype(_BF)
    ivp_np = np.empty((NIC, 128, 8 * 128), dtype=_BF)
    for g in range(NIC):
        par, tc = g % 2, g // 2
        for m in range(8):
            M = mats[(par, 0 if m < 4 else 1)]
            jc = m % 4
            ivp_np[g, :, m * 128:(m + 1) * 128] = \
                M[jc * 128:(jc + 1) * 128, tc * 128:(tc + 1) * 128]
    return dict(cfh=np.ascontiguousarray(cfh_np),
                cfl=np.ascontiguousarray(cl),
                sfh=np.ascontiguousarray(sh),
                sfl=np.ascontiguousarray(sl),
                ivp=np.ascontiguousarray(ivp_np))


def _fold_signals(xb):
    x64 = xb.astype(np.float64)
    xs = np.zeros((1024, D))
    xd = np.zeros((1024, D))
    xs[0] = x64[0]
    xs[1:] = x64[1:1024] + x64[2048:1024:-1]
    xd[1:] = x64[1:1024] - x64[2048:1024:-1]
    xs_h, xs_l = _split_hilo(xs.astype(np.float32))
    xd_h, xd_l = _split_hilo(xd.astype(np.float32))
    xb_h, xb_l = _split_hilo(x64[1024].astype(np.float32)[None, :])
    xsig = np.zeros((17 * 128, D), dtype=_BF)
    xsig[0:1024] = xs_h
    xsig[1024] = xb_h[0]
    xsig[1025] = xb_l[0]
    xsig[1152:2176] = xs_l
    xdsig = np.empty((16 * 128, D), dtype=_BF)
    xdsig[0:1024] = xd_h
    xdsig[1024:2048] = xd_l
    return xsig, xdsig


_CONSTS = None
LAST_EXEC_NS = None
LAST_RES = None
TRACE = False


def kernel(input_tensor: np.ndarray) -> np.ndarray:
    from concourse.bass_utils import run_bass_kernel_spmd

    global _CONSTS
    if _CONSTS is None:
        _CONSTS = _make_constants()

    x = np.asarray(input_tensor, dtype=np.float32)
    B = x.shape[0]
    assert x.shape == (B, T, D)

    nc = bass.Bass("TRN2", target_bir_lowering=False)
    build_kernel(nc)

    in_maps = []
    for b in range(B):
        xsig, xdsig = _fold_signals(x[b])
        in_maps.append({"xsig": xsig, "xdsig": xdsig, **_CONSTS})

    global LAST_EXEC_NS, LAST_RES
    res = run_bass_kernel_spmd(nc, in_maps, core_ids=list(range(B)), trace=TRACE)
    LAST_EXEC_NS = res.exec_time_ns
    LAST_RES = res
    return np.stack([res.results[b]["out"] for b in range(B)], axis=0)


if __name__ == "__main__":
    rng = np.random.default_rng(0)
    x = rng.standard_normal((8, T, D), dtype=np.float32)
    y = kernel(input_tensor=x)
    print("out", y.shape, y.dtype)


# revision 84
# speedup vs baseline: 87.1938x; 87.1938x over previous
"""FourierLayer TRN2 kernel: folded DFT -> top-6 mask -> folded inverse DFT.

Contract: kernel(input_tensor=(8,2048,512) f32) -> (8,2048,512) f32.
Each of the 8 NeuronCores processes one batch element (data-parallel over
batch; no cross-core communication).

Math (per core, T=2048, D=512, top-6 of 1023 rfft bins):
  Time-fold (host): xs[t']=x[t']+x[2048-t'], xd[t']=x[t']-x[2048-t']
  halves the forward DFT:  Re_k = sum_t' CF[t',k] xs[t'] (+(-1)^k x[1024]),
  Im_k = sum_t' SF[t',k] xd[t'].  Forward runs 4 bf16 hi/lo products
  (Ch@xh + Ch@xl + Cl@xh + Cl@xl) for ~fp32 magnitudes; on the fixed
  harness input this reproduces the reference top-6 selection exactly
  (3 products leave ~1e-6 systematic noise that flips near-tie channels).
  Frequency columns are ordered [k=1..512, k=1023..513, k=1024(zero)] so
  the inverse frequency-fold A+/- = R2[0:512] +/- R2[512:1024] (B-/+ for
  I2) is an aligned DVE add; even/odd output rows then come from two
  half-size inverse transforms (bf16 hi-only matrices).

Raw bass with manual semaphores (TileContext auto-sync emits >2 sync
commands per instruction, which this toolchain's walrus rejects). All DMAs
go through gpsimd/SWDGE. SDMA completions are NOT ordered across transfers
(verified by CoreSim's race detector and by nondeterministic corruption on
HW with unequal transfer sizes), so every DMA wait is a full-set wait on a
dedicated semaphore (or on a ring semaphore with at most one outstanding
transfer). The PE drops to a low-frequency pstate after any SEQ stall and
re-ramps over ~3us of dispatch, so the forward runs in four kc-half phases
with no mid-stream waits, and fp32 dummy matmuls keep the PE busy across
the initial DMA wait and the theta/mask/fold chain.
"""

from contextlib import ExitStack

import numpy as np
import ml_dtypes

import concourse.bass as bass
import concourse.mybir as mybir

BF16 = mybir.dt.bfloat16
F32 = mybir.dt.float32
AF = mybir.ActivationFunctionType
ALU = mybir.AluOpType

T = 2048          # time length
D = 512           # channels
KF = 1024         # padded frequency count (col c -> k=perm[c]; col KF-1 zero)
NKC = KF // 128   # 8 freq chunks
NDC = D // 128    # 4 channel chunks
TOPK = 6
NFC = 9           # forward folded time chunks for Re (8 + boundary)
NIC = 16          # inverse groups: (t' chunk 0..7) x (parity)
RSI = 4           # inverse stream ring slots

# ---- semaphore schedules (python-side bookkeeping) ----
# DMA semaphores (16 per transfer). SDMA completions are NOT ordered across
# transfers (CoreSim race detector + observed HW nondeterminism), so every
# wait must be a full-set wait on its own semaphore:
#   s_lx {xsig}, s_lc {cfh}, s_lc2 {cfl}, s_ld {xdsig}, s_ls {sfh},
#   s_ls2 {sfl}: single transfers, wait >= 16.
#   s_iv[par] {iv loads j with j%2==par}: ring gating keeps <=1 outstanding
#     per sem; group g waits s_iv[g%2] >= 16*(g//2+1).
#   s_ot[par] {out stores g%2==par}: <=1 outstanding; ACT evict g waits
#     s_ot[g%2] >= 16*(g//2).
# The forward runs in four kc-half phases so evictions overlap compute and
# the PE never blocks mid-stream (each blocking wait re-pins the PE to its
# low-frequency pstate for ~3us of dispatch):
#   A: Re kc0..3 -> banks0..3   B: Im kc0..3 -> banks4..7
#   C: Re kc4..7 -> banks0..3   D: Im kc4..7 -> banks4..7
# fp32 dummy matmuls pre-ramp the PE across the initial DMA wait and bridge
# the theta/mask/fold gaps before the inverse.
# s_pe:  fwd A 1..33, B 34..65, C 66..98, D 99..130; transposes 131..162;
#        ones-bcast 163; inverse groups 164..179
# s_act: ReL evicts 1..4; ImL 5..8; ReH 9..12; ImH 13..16; transpose
#        copies 17..48; thb 49; out evicts 50..65
# s_dve: magL 1; magH kc4..7 2..5; max8 6..9; mask muls 10..13; folds 14..17
# s_th: 4 theta-row DMAs (full-set wait >= 64)
# s_pool: scratch 1; ones 2; identity 3
# Transposes/copies run in two kc-half waves (kc0..3 for all dc first) so
# the magH chain overlaps wave-1 PE work; masks are computed in place in
# mag (it is dead after the transposes).


def build_kernel(nc: bass.Bass):
    xsig = nc.dram_tensor("xsig", (17 * 128, D), BF16, kind="ExternalInput")
    xdsig = nc.dram_tensor("xdsig", (16 * 128, D), BF16, kind="ExternalInput")
    cfh = nc.dram_tensor("cfh", (NFC * 128, KF), BF16, kind="ExternalInput")
    cfl = nc.dram_tensor("cfl", (8 * 128, KF), BF16, kind="ExternalInput")
    sfh = nc.dram_tensor("sfh", (8 * 128, KF), BF16, kind="ExternalInput")
    sfl = nc.dram_tensor("sfl", (8 * 128, KF), BF16, kind="ExternalInput")
    # inverse blocks: group g=(tc*2+par): [p, m*128+u] = M[(m%4)*128+p,
    # tc*128+u] with M = CI(par) for m<4, SI(par) for m>=4
    ivp = nc.dram_tensor("ivp", (NIC, 128, 8 * 128), BF16, kind="ExternalInput")
    out = nc.dram_tensor("out", (T, D), F32, kind="ExternalOutput")

    with ExitStack() as ctx:
        def sb(name, shape, dtype):
            return ctx.enter_context(nc.sbuf_tensor(name, shape, dtype))

        xs_sb = sb("xs_sb", [128, 17 * D], BF16)
        xd_sb = sb("xd_sb", [128, 16 * D], BF16)
        cfh_sb = sb("cfh_sb", [128, NFC * KF], BF16)
        cfl_sb = sb("cfl_sb", [128, 8 * KF], BF16)
        sfh_sb = sb("sfh_sb", [128, 8 * KF], BF16)
        sfl_sb = sb("sfl_sb", [128, 8 * KF], BF16)
        iv_sb = sb("iv_sb", [128, RSI * 8 * 128], BF16)  # RSI-slot ring
        r2 = sb("r2", [128, NKC * D], F32)
        i2 = sb("i2", [128, NKC * D], F32)
        mag = sb("mag", [128, NKC * D], F32)
        mag_t = sb("mag_t", [128, NDC * KF], F32)
        ap_h = sb("ap_h", [128, 4 * D], BF16)
        am_h = sb("am_h", [128, 4 * D], BF16)
        bp_h = sb("bp_h", [128, 4 * D], BF16)
        bm_h = sb("bm_h", [128, 4 * D], BF16)
        m8 = sb("m8", [128, NDC * 8], F32)
        trows = [sb(f"trow{i}", [1, 128], F32) for i in range(NDC)]
        thb = sb("thb", [128, D], F32)
        ones = sb("ones", [1, 128], F32)
        ident = sb("ident", [128, 128], F32)
        msk4 = sb("msk4", [128, 2 * D], F32)
        scr = sb("scr", [128, D], F32)
        ot_sb = sb("ot_sb", [128, 4 * D], F32)
        banks = [ctx.enter_context(nc.psum_tensor(f"pb{i}", [128, D], F32))
                 for i in range(8)]
        pb5 = banks[5]
        s_lx = ctx.enter_context(nc.semaphore())
        s_lc = ctx.enter_context(nc.semaphore())
        s_lc2 = ctx.enter_context(nc.semaphore())
        s_ld = ctx.enter_context(nc.semaphore())
        s_ls = ctx.enter_context(nc.semaphore())
        s_ls2 = ctx.enter_context(nc.semaphore())
        s_iv = [ctx.enter_context(nc.semaphore(name=f"s_iv{i}"))
                for i in range(RSI)]
        s_th = ctx.enter_context(nc.semaphore())
        s_ot = [ctx.enter_context(nc.semaphore(name=f"s_ot{i}"))
                for i in range(2)]
        s_pe = ctx.enter_context(nc.semaphore())
        s_act = ctx.enter_context(nc.semaphore())
        s_dve = ctx.enter_context(nc.semaphore())
        s_pool = ctx.enter_context(nc.semaphore())
        block = ctx.enter_context(nc.Block())

        NPRE = 10       # pre-ramp fp32 dummies spanning the first DMA wait
        NBRIDGE = 27    # bridge dummies spanning the mask/fold gap

        @block.gpsimd
        def _(gpsimd):
            gpsimd.memset(scr[:], 1.0).then_inc(s_pool, 1)
            gpsimd.memset(ones[:], 1.0).then_inc(s_pool, 1)
            gpsimd.memset(ident[:], 0.0)
            gpsimd.drain()
            nc.gpsimd.affine_select(
                out=ident[:], in_=ident[:],
                compare_op=ALU.not_equal, fill=1.0, base=0,
                pattern=[[-1, 128]], channel_multiplier=1,
            ).then_inc(s_pool, 1)
            # bulk loads: one semaphore per transfer (full-set waits only)
            gpsimd.dma_start(
                xs_sb[:, :],
                xsig[:].rearrange("(a p) d -> p a d", p=128)).then_inc(s_lx, 16)
            gpsimd.dma_start(
                cfh_sb[:, :],
                cfh[:].rearrange("(a p) c -> p a c", p=128)).then_inc(s_lc, 16)
            gpsimd.dma_start(
                cfl_sb[:, :],
                cfl[:].rearrange("(a p) c -> p a c", p=128)).then_inc(s_lc2, 16)
            gpsimd.dma_start(
                xd_sb[:, :],
                xdsig[:].rearrange("(a p) d -> p a d", p=128)).then_inc(s_ld, 16)
            gpsimd.dma_start(
                sfh_sb[:, :],
                sfh[:].rearrange("(a p) c -> p a c", p=128)).then_inc(s_ls, 16)
            gpsimd.dma_start(
                sfl_sb[:, :],
                sfl[:].rearrange("(a p) c -> p a c", p=128)).then_inc(s_ls2, 16)
            # inverse stream: pair-batched loads (groups 2p,2p+1 -> adjacent
            # ring slots); prefetch the first two pairs
            for p in range(2):
                gpsimd.dma_start(
                    iv_sb[:, (2 * p % RSI) * KF:(2 * p % RSI + 2) * KF],
                    ivp[2 * p:2 * p + 2, :, :].rearrange("a p c -> p a c"),
                ).then_inc(s_iv[p % 2], 16)
            # theta rows: the four 6th-largest columns of m8 -> one (1,512)
            # row (PE cannot transpose single columns; matmul rhs must sit
            # at partition 0)
            for dc in range(NDC):
                gpsimd.wait_ge(s_dve, 6 + dc)   # max8 for dc done
                gpsimd.dma_start(
                    trows[dc][:, :],
                    m8[:, dc * 8 + TOPK - 1: dc * 8 + TOPK]).then_inc(s_th, 16)
            # remaining inverse stream pairs interleaved with output stores
            # (out pair q covers groups 2q,2q+1 from adjacent ot slots)
            out_v = out[:].rearrange("(a two) d -> two a d", two=2)
            NP = NIC // 2
            for j in range(2, NP + 2):
                if j < NP:
                    # ring pair slots freed once pair j-2's groups finish
                    gpsimd.wait_ge(s_pe, 164 + 2 * (j - 2) + 1)
                    gpsimd.dma_start(
                        iv_sb[:, (2 * j % RSI) * KF:(2 * j % RSI + 2) * KF],
                        ivp[2 * j:2 * j + 2, :, :].rearrange("a p c -> p a c"),
                    ).then_inc(s_iv[j % 2], 16)
                q = j - 2
                gpsimd.wait_ge(s_act, 51 + 2 * q)   # both evicts done
                off = (2 * q % 4) * D
                gpsimd.dma_start(
                    out[q * 256:(q + 1) * 256, :].rearrange(
                        "(p two) d -> p two d", two=2),
                    ot_sb[:, off:off + 2 * D],
                ).then_inc(s_ot[q % 2], 16)
            drains = [(s_lx, 16), (s_lc, 16), (s_lc2, 16), (s_ld, 16),
                      (s_ls, 16), (s_ls2, 16), (s_th, 64),
                      (s_iv[0], 64), (s_iv[1], 64),
                      (s_ot[0], 64), (s_ot[1], 64)]
            for sem, tot in drains:
                gpsimd.wait_ge(sem, tot)

        @block.tensor
        def _(tensor):
            # pre-ramp: fp32 dummies keep the PE busy across the DMA wait
            tensor.wait_ge(s_pool, 1)
            for _ in range(NPRE):
                nc.tensor.matmul(banks[0][:], scr[:, 0:128], scr[:, :],
                                 start=True, stop=True)
            # forward DFT in four kc-half phases
            re_prods = [
                (cfh_sb, xs_sb, 0, 9, (s_lx, s_lc)),    # hh incl boundary
                (cfh_sb, xs_sb, 9, 8, ()),              # hl
                (cfl_sb, xs_sb, 0, 8, (s_lc2,)),        # lh
                (cfl_sb, xs_sb, 9, 8, ()),              # ll
            ]
            im_prods = [
                (sfh_sb, xd_sb, 0, 8, (s_ld, s_ls)),
                (sfh_sb, xd_sb, 8, 8, ()),
                (sfl_sb, xd_sb, 0, 8, (s_ls2,)),
                (sfl_sb, xd_sb, 8, 8, ()),
            ]
            phases = [
                (re_prods, 0, 0, None),     # A: Re kc0..3 -> banks0..3
                (im_prods, 0, 4, None),     # B: Im kc0..3 -> banks4..7
                (re_prods, 4, 0, 4),        # C: Re kc4..7 (banks evicted)
                (im_prods, 4, 4, 8),        # D: Im kc4..7
            ]
            for prods, klo, bbase, act_wait in phases:
                if act_wait is not None:
                    tensor.wait_ge(s_act, act_wait)
                ng = sum(p[3] for p in prods)
                g = 0
                for mat, rhs, r0, n, sems in prods:
                    for sem in sems:
                        tensor.wait_ge(sem, 16)
                    for tc in range(n):
                        rc = rhs[:, (r0 + tc) * D:(r0 + tc + 1) * D]
                        for ki in range(4):
                            kc = klo + ki
                            mm = nc.tensor.matmul(
                                banks[bbase + ki][:],
                                mat[:, tc * KF + kc * 128:
                                    tc * KF + (kc + 1) * 128],
                                rc, start=(g == 0), stop=(g == ng - 1))
                            if ki == 3:
                                mm.then_inc(s_pe, 1)
                        g += 1
            # mag transposes in two kc-half waves (kc0..3 first: their mag is
            # ready early; magH overlaps wave 1); 8-bank rotation
            tensor.wait_ge(s_pool, 3)
            for j in range(32):
                half, dc, ki = j // 16, (j % 16) // 4, j % 4
                kc = half * 4 + ki
                tensor.wait_ge(s_dve, 1 if half == 0 else 2 + ki)
                tensor.wait_ge(s_act, 9 + j)
                nc.tensor.transpose(
                    banks[j % 8][:, 0:128],
                    mag[:, kc * D + dc * 128: kc * D + (dc + 1) * 128],
                    ident[:]).then_inc(s_pe, 1)
            # bridge dummies across the copies/max8 gap (bank1: its transpose
            # copies are done once s_act >= 42; bank0 stays free for theta)
            tensor.wait_ge(s_act, 42)
            for _ in range(5):
                nc.tensor.matmul(banks[1][:], scr[:, 0:128], scr[:, :],
                                 start=True, stop=True)
            # ones-broadcast (fp32, exact): trow rows -> thb psum (bank5)
            tensor.wait_ge(s_th, 64)  # theta row DMAs done
            for dc in range(NDC):
                mm = nc.tensor.matmul(pb5[:, dc * 128:(dc + 1) * 128],
                                      ones[:], trows[dc][:],
                                      start=(dc == 0), stop=(dc == NDC - 1))
                if dc == NDC - 1:
                    mm.then_inc(s_pe, 1)
            # bridge dummies across the mask/fold gap
            for _ in range(NBRIDGE):
                nc.tensor.matmul(banks[1][:], scr[:, 0:128], scr[:, :],
                                 start=True, stop=True)
            # inverse DFT: 16 groups (t' chunk x parity)
            tensor.wait_ge(s_dve, 17)  # folds done
            for g in range(NIC):
                par = g % 2
                tensor.wait_ge(s_iv[(g // 2) % 2], 16 * (g // 4 + 1))
                if g >= 4:
                    tensor.wait_ge(s_act, 46 + g)  # bank evicted
                bank = banks[g % 4]
                sl0 = (g % RSI) * KF
                ca, cb = (ap_h, bp_h) if par == 0 else (am_h, bm_h)
                for jc in range(4):
                    nc.tensor.matmul(
                        bank[:], iv_sb[:, sl0 + jc * 128: sl0 + (jc + 1) * 128],
                        ca[:, jc * D:(jc + 1) * D],
                        start=(jc == 0), stop=False)
                    mm = nc.tensor.matmul(
                        bank[:],
                        iv_sb[:, sl0 + (4 + jc) * 128: sl0 + (5 + jc) * 128],
                        cb[:, jc * D:(jc + 1) * D],
                        start=False, stop=(jc == 3))
                    if jc == 3:
                        mm.then_inc(s_pe, 1)

        @block.scalar
        def _(scalar):
            # phase evictions; x2 scale folds the conjugate doubling
            # (A: r2 kc0..3, B: i2 kc0..3, C: r2 kc4..7, D: i2 kc4..7)
            for pe_w, dst, klo, bbase in ((33, r2, 0, 0), (65, i2, 0, 4),
                                          (98, r2, 4, 0), (130, i2, 4, 4)):
                scalar.wait_ge(s_pe, pe_w)
                for ki in range(4):
                    kc = klo + ki
                    nc.scalar.activation(dst[:, kc * D:(kc + 1) * D],
                                         banks[bbase + ki][:],
                                         AF.Copy, scale=2.0).then_inc(s_act, 1)
            # transpose copies (same two-wave order as the transposes)
            for j in range(32):
                half, dc, ki = j // 16, (j % 16) // 4, j % 4
                kc = half * 4 + ki
                scalar.wait_ge(s_pe, 131 + j)
                nc.scalar.activation(
                    mag_t[:, dc * KF + kc * 128: dc * KF + (kc + 1) * 128],
                    banks[j % 8][:, 0:128], AF.Copy).then_inc(s_act, 1)
            # thb copy
            scalar.wait_ge(s_pe, 163)
            nc.scalar.activation(thb[:], pb5[:], AF.Copy).then_inc(s_act, 1)
            # inverse evictions into the 4-slot out ring
            for g in range(NIC):
                scalar.wait_ge(s_pe, 164 + g)
                if g >= 4:
                    # slot g%4 reused from pair (g//2 - 2)
                    q = g // 2 - 2
                    scalar.wait_ge(s_ot[q % 2], 16 * (q // 2 + 1))
                nc.scalar.activation(
                    ot_sb[:, (g % 4) * D:(g % 4 + 1) * D],
                    banks[g % 4][:], AF.Copy).then_inc(s_act, 1)

        @block.vector
        def _(vector):
            # magnitudes: low half in one wide op sequence; high half per-kc
            # so each chunk lands right after its ImH eviction (msk4 is the
            # i2^2 scratch before the mask phase)
            H = 4 * D
            vector.wait_ge(s_act, 8)
            for qh in range(2):
                qsl = slice(qh * H // 2, (qh + 1) * H // 2)
                nc.vector.tensor_tensor(mag[:, qsl], r2[:, qsl], r2[:, qsl],
                                        ALU.mult)
                nc.vector.tensor_tensor(msk4[:], i2[:, qsl], i2[:, qsl],
                                        ALU.mult)
                mm = nc.vector.tensor_tensor(mag[:, qsl], mag[:, qsl],
                                             msk4[:], ALU.add)
                if qh == 1:
                    mm.then_inc(s_dve, 1)
            for kc in range(4, NKC):
                vector.wait_ge(s_act, 9 + kc)   # ImH evict for kc done
                csl = slice(kc * D, (kc + 1) * D)
                nc.vector.tensor_tensor(mag[:, csl], r2[:, csl], r2[:, csl],
                                        ALU.mult)
                nc.vector.tensor_tensor(msk4[:, 0:D], i2[:, csl], i2[:, csl],
                                        ALU.mult)
                nc.vector.tensor_tensor(mag[:, csl], mag[:, csl],
                                        msk4[:, 0:D],
                                        ALU.add).then_inc(s_dve, 1)
            # top-8 + 6th-largest per channel (copies land in wave order:
            # dc's last copy is j=19+4dc -> s_act 36+4dc)
            for dc in range(NDC):
                vector.wait_ge(s_act, 36 + 4 * dc)
                nc.vector.max(out=m8[:, dc * 8:(dc + 1) * 8],
                              in_=mag_t[:, dc * KF:(dc + 1) * KF]).then_inc(s_dve, 1)
            # mask in place in mag, then wide mask-muls and folds
            vector.wait_ge(s_act, 49)
            for kc in range(NKC):
                csl = slice(kc * D, (kc + 1) * D)
                nc.vector.tensor_tensor(mag[:, csl], mag[:, csl], thb[:],
                                        ALU.is_ge)
            for src in (r2, i2):
                for half in range(2):
                    hsl = slice(half * H, (half + 1) * H)
                    nc.vector.tensor_tensor(src[:, hsl], src[:, hsl],
                                            mag[:, hsl],
                                            ALU.mult).then_inc(s_dve, 1)
            # frequency folds (bf16 out), halves = [128, 4*D] slices
            nc.vector.tensor_tensor(ap_h[:], r2[:, 0:H], r2[:, H:2 * H],
                                    ALU.add).then_inc(s_dve, 1)
            nc.vector.tensor_tensor(am_h[:], r2[:, 0:H], r2[:, H:2 * H],
                                    ALU.subtract).then_inc(s_dve, 1)
            nc.vector.tensor_tensor(bp_h[:], i2[:, 0:H], i2[:, H:2 * H],
                                    ALU.subtract).then_inc(s_dve, 1)
            nc.vector.tensor_tensor(bm_h[:], i2[:, 0:H], i2[:, H:2 * H],
                                    ALU.add).then_inc(s_dve, 1)


# ---------------- host side ----------------

_BF = ml_dtypes.bfloat16


def _split_hilo(a32):
    hi = a32.astype(_BF)
    lo = (a32.astype(np.float32) - hi.astype(np.float32)).astype(_BF)
    return hi, lo


def _perm():
    p = np.empty(KF, dtype=np.int64)
    p[0:512] = np.arange(1, 513)
    p[512:1023] = np.arange(1023, 512, -1)
    p[1023] = 1024
    return p


def _make_constants():
    perm = _perm()
    tp = np.arange(1024, dtype=np.float64)[:, None]
    kk = perm[None, :].astype(np.float64)
    ang = 2.0 * np.pi * tp * kk / T
    CF = np.cos(ang)
    SF = -np.sin(ang)
    CF[:, KF - 1] = 0.0
    SF[:, KF - 1] = 0.0
    SF[0, :] = 0.0
    ch, cl = _split_hilo(CF.astype(np.float32))
    sh, sl = _split_hilo(SF.astype(np.float32))
    bnd = np.cos(np.pi * perm)
    bnd[KF - 1] = 0.0
    cfh_np = np.zeros((NFC * 128, KF), dtype=_BF)
    cfh_np[0:1024] = ch
    cfh_np[1024] = bnd.astype(_BF)
    cfh_np[1025] = bnd.astype(_BF)

    jj = np.arange(512, dtype=np.float64)[:, None] + 1.0
    te = 2.0 * np.arange(1024, dtype=np.float64)[None, :]
    mats = {}
    for par, toff in ((0, 0.0), (1, 1.0)):
        a = 2.0 * np.pi * jj * (te + toff) / T
        mats[(par, 0)] = np.cos(a).astype(np.float32).astype(_BF)
        mats[(par, 1)] = (-np.sin(a)).astype(np.float32).astype(_BF)
    ivp_np = np.empty((NIC, 128, 8 * 128), dtype=_BF)
    for g in range(NIC):
        par, tc = g % 2, g // 2
        for m in range(8):
            M = mats[(par, 0 if m < 4 else 1)]
            jc = m % 4
            ivp_np[g, :, m * 128:(m + 1) * 128] = \
                M[jc * 128:(jc + 1) * 128, tc * 128:(tc + 1) * 128]
    return dict(cfh=np.ascontiguousarray(cfh_np),
                cfl=np.ascontiguousarray(cl),
                sfh=np.ascontiguousarray(sh),
                sfl=np.ascontiguousarray(sl),
                ivp=np.ascontiguousarray(ivp_np))


def _fold_signals(xb):
    x64 = xb.astype(np.float64)
    xs = np.zeros((1024, D))
    xd = np.zeros((1024, D))
    xs[0] = x64[0]
    xs[1:] = x64[1:1024] + x64[2048:1024:-1]
    xd[1:] = x64[1:1024] - x64[2048:1024:-1]
    xs_h, xs_l = _split_hilo(xs.astype(np.float32))
    xd_h, xd_l = _split_hilo(xd.astype(np.float32))
    xb_h, xb_l = _split_hilo(x64[1024].astype(np.float32)[None, :])
    xsig = np.zeros((17 * 128, D), dtype=_BF)
    xsig[0:1024] = xs_h
    xsig[1024] = xb_h[0]
    xsig[1025] = xb_l[0]
    xsig[1152:2176] = xs_l
    xdsig = np.empty((16 * 128, D), dtype=_BF)
    xdsig[0:1024] = xd_h
    xdsig[1024:2048] = xd_l
    return xsig, xdsig


_CONSTS = None
LAST_EXEC_NS = None
LAST_RES = None
TRACE = False


def kernel(input_tensor: np.ndarray) -> np.ndarray:
    from concourse.bass_utils import run_bass_kernel_spmd

    global _CONSTS
    if _CONSTS is None:
        _CONSTS = _make_constants()

    x = np.asarray(input_tensor, dtype=np.float32)
    B = x.shape[0]
    assert x.shape == (B, T, D)

    nc = bass.Bass("TRN2", target_bir_lowering=False)
    build_kernel(nc)

    in_maps = []
    for b in range(B):
        xsig, xdsig = _fold_signals(x[b])
        in_maps.append({"xsig": xsig, "xdsig": xdsig, **_CONSTS})

    global LAST_EXEC_NS, LAST_RES
    res = run_bass_kernel_spmd(nc, in_maps, core_ids=list(range(B)), trace=TRACE)
    LAST_EXEC_NS = res.exec_time_ns
    LAST_RES = res
    return np.stack([res.results[b]["out"] for b in range(B)], axis=0)


if __name__ == "__main__":
    rng = np.random.default_rng(0)
    x = rng.standard_normal((8, T, D), dtype=np.float32)
    y = kernel(input_tensor=x)
    print("out", y.shape, y.dtype)
